# revision 19
# baseline (speedup 1.0000x reference)
"""Trainium2 Bass kernel for LoRACrossAttnProcessor (v7, bf16, mega-stream).

Strategy:
- Host: fold LoRA (W_eff = W + up @ down, exact in f64), permute the qkv
  output channels (rows of Wq/Wk/Wv, cols of Wo) so each head owns one full
  128-row tile (tiles 0-7) plus a 32-row slice of the shared leftover tiles
  (8-9).  Pre-transpose to partition-major DRAM layouts (one contiguous
  chunk per SBUF partition) and cast to bf16 on the host.
- Shard: data-parallel over batch, 2 batch items per core, 8 cores.
- Device (per core, bf16 matmuls, fp32 PSUM):
    K.T tiles  = Wk_p @ E.T      [128, 154] per tile (both batches at once)
    V          = E @ Wv_p.T      [77, 1024] main + [77, 8, 33] leftover+ones
    Q.T        = Wq_p @ X.T      [128, 10, 1024] bf16 per batch
    per (batch, head, st-chunk):
      scores.T = kt/ktm MMs -> [77, 512] fp32 PSUM
      exps     = exp(scores.T * scale) -> bf16  (ACT only; kept FIFO-clean)
      A.T main = V[:, head-tile] @ exps          [128, 512]
      A.T left = [V_left | 1] @ exps -> [33, 512]; row 32 = sumexp
      recip -> bf16 (DVE), partition-broadcast bf16 (Pool), STT -> at
    O.T        = Wo_p @ A.T     [128, 1024] f32 staged, contiguous DMA out
- HBM-deadline-aware staging: wq is loaded in five column chunks (separate
  tiles) so Q-proj units can start ~20us in while later chunks stream; wv
  loads in two chunks with the V projection emitted mid-stream; attention
  backs trail their fronts by two windows so V is ready for head 0.
- Single mega-stream emission keeps the PE busy through both batches'
  attention; a shared 8-slot PSUM pool makes bank-reuse distance a full
  head-window.
- Host: transpose O.T -> O, add bo.
"""

import numpy as np
import ml_dtypes
from contextlib import ExitStack

import concourse.bass as bass
import concourse.mybir as mybir
import concourse.tile as tile
from concourse import bacc
from concourse.bass_utils import run_bass_kernel_spmd

F32 = mybir.dt.float32
BF16 = mybir.dt.bfloat16
AF = mybir.ActivationFunctionType
MULT = mybir.AluOpType.mult

H = 8
B, S, C = 16, 1024, 1280
SENC, CENC = 77, 1024
D = C // H  # 160
NCORES = 8
BPC = B // NCORES  # 2
P = 128
NCI_Q = C // P  # 10
NCI_KV = CENC // P  # 8
NCO = C // P  # 10
EW = 2 * SENC  # 154, both batches' encoder tokens side by side
ATTN_SCALE = 1.0 / float(np.sqrt(D))
ST = (slice(0, 512), slice(512, 1024))


def head_perm():
    """New channel order: head h gets rows [128h,128h+128) (its first 128
    dims) and rows [1024+32h, 1024+32h+32) (its last 32 dims)."""
    perm = []
    for h in range(H):
        perm.extend(range(D * h, D * h + P))
    for h in range(H):
        perm.extend(range(D * h + P, D * h + D))
    return np.asarray(perm)


def build():
    nc = bacc.Bacc("TRN2", target_bir_lowering=False, debug=False)
    xt_d = nc.dram_tensor("xt", [BPC, P, NCI_Q, S], BF16, kind="ExternalInput")
    et_d = nc.dram_tensor("et", [P, NCI_KV, EW], BF16, kind="ExternalInput")
    wk_d = nc.dram_tensor("wk", [2, P, NCI_KV, 640], BF16, kind="ExternalInput")
    wva_d = nc.dram_tensor("wva", [P, NCI_KV, 512], BF16, kind="ExternalInput")
    wvb_d = nc.dram_tensor("wvb", [P, NCI_KV, 768], BF16, kind="ExternalInput")
    wq_d = nc.dram_tensor("wq", [5, P, NCI_Q, 256], BF16, kind="ExternalInput")
    wo_d = nc.dram_tensor("wo", [P, NCI_Q, C], BF16, kind="ExternalInput")
    otd_d = nc.dram_tensor("otd", [BPC, C, S], F32, kind="ExternalOutput")

    with tile.TileContext(nc) as tc, ExitStack() as ctx:
        wpool = ctx.enter_context(tc.tile_pool(name="wpool", bufs=1))  # wo only
        apool = ctx.enter_context(tc.tile_pool(name="apool", bufs=4))
        persist = ctx.enter_context(tc.tile_pool(name="persist", bufs=1))
        expp = ctx.enter_context(tc.tile_pool(name="expp", bufs=8))
        bcp = ctx.enter_context(tc.tile_pool(name="bcp", bufs=2))
        recp = ctx.enter_context(tc.tile_pool(name="recp", bufs=2))
        lost = ctx.enter_context(tc.tile_pool(name="lost", bufs=2))
        ostg = ctx.enter_context(tc.tile_pool(name="ostg", bufs=2))
        psp = ctx.enter_context(tc.tile_pool(name="psp", bufs=8, space="PSUM"))

        # ---- persistent buffers ----
        kt = [
            persist.tile([P, EW], BF16, tag=f"kt{t}", name=f"kt{t}")
            for t in range(H)
        ]
        ktm = [
            [
                persist.tile(
                    [P, EW], BF16, tag=f"ktm{i}_{m}", name=f"ktm{i}_{m}"
                )
                for m in range(4)
            ]
            for i in range(2)
        ]
        for i in range(2):
            for m in range(4):
                nc.vector.memset(ktm[i][m], 0.0)
        v_nat = [
            persist.tile([SENC, CENC], BF16, tag=f"vnat{b}", name=f"vnat{b}")
            for b in range(BPC)
        ]
        # [V_leftover(32) | ones] per head: row 32 of the A.T-leftover matmul
        # output is then the softmax denominator.
        vlo = [
            persist.tile([SENC, H, 33], BF16, tag=f"vlo{b}", name=f"vlo{b}")
            for b in range(BPC)
        ]
        for b in range(BPC):
            nc.vector.memset(vlo[b][:, :, 32:33], 1.0)
        et = persist.tile([P, NCI_KV, EW], BF16, tag="et")

        # ---- input DMAs, HBM-deadline order ----
        # sync ring:   wk, wq chunks (c4 first: Q cols for heads' tiles 8,9)
        # scalar ring: et, xt0, wva, wvb, xt1, wo
        wka = apool.tile([P, NCI_KV, 640], BF16, tag="act", name="wka")
        nc.sync.dma_start(out=wka, in_=wk_d.ap()[0])
        wkb = apool.tile([P, NCI_KV, 640], BF16, tag="act", name="wkb")
        nc.sync.dma_start(out=wkb, in_=wk_d.ap()[1])
        wqc = [
            persist.tile([P, NCI_Q, 256], BF16, tag=f"wqc{j}", name=f"wqc{j}")
            for j in range(5)
        ]
        for j in (4, 0, 1, 2, 3):
            nc.sync.dma_start(out=wqc[j], in_=wq_d.ap()[j])
        nc.scalar.dma_start(out=et, in_=et_d.ap())
        xt = []
        for b in range(BPC):
            x = apool.tile([P, NCI_Q, S], BF16, tag="act", name=f"xt{b}")
            xt.append(x)
        nc.scalar.dma_start(out=xt[0], in_=xt_d.ap()[0])
        wva = persist.tile([P, NCI_KV, 512], BF16, tag="wva")
        nc.scalar.dma_start(out=wva, in_=wva_d.ap())
        wvb = persist.tile([P, NCI_KV, 768], BF16, tag="wvb")
        nc.scalar.dma_start(out=wvb, in_=wvb_d.ap())
        nc.scalar.dma_start(out=xt[1], in_=xt_d.ap()[1])

        # ---- K.T projection: both batches at once ----
        for t in range(NCO):
            ps = psp.tile([P, EW], F32, tag="ps", name=f"psk{t}")
            wkc = wka if t < 5 else wkb
            tt = t if t < 5 else t - 5
            for ci in range(NCI_KV):
                nc.tensor.matmul(
                    ps,
                    wkc[:, ci, tt * P : (tt + 1) * P],
                    et[:, ci, :],
                    start=(ci == 0),
                    stop=(ci == NCI_KV - 1),
                )
            if t < H:
                nc.vector.tensor_copy(out=kt[t], in_=ps)
            else:
                for m in range(4):
                    nc.vector.tensor_copy(
                        out=ktm[t - H][m][32 * m : 32 * m + 32, :],
                        in_=ps[32 * m : 32 * m + 32, :],
                    )

        wo = wpool.tile([P, NCI_Q, C], BF16, tag="w", name="wo")
        nc.scalar.dma_start(out=wo, in_=wo_d.ap())

        def v_proj():
            VCH = [(0, 512), (512, 512), (1024, 256)]
            VW = [
                lambda ci: wva[:, ci, 0:512],
                lambda ci: wvb[:, ci, 0:512],
                lambda ci: wvb[:, ci, 512:768],
            ]
            for b in range(BPC):
                for j, (cc, w) in enumerate(VCH):
                    ps = psp.tile(
                        [SENC, 512], F32, tag="ps", name=f"psv{b}_{j}"
                    )
                    for ci in range(NCI_KV):
                        nc.tensor.matmul(
                            ps[:, :w],
                            et[:, ci, b * SENC : (b + 1) * SENC],
                            VW[j](ci),
                            start=(ci == 0),
                            stop=(ci == NCI_KV - 1),
                        )
                    if j < 2:
                        nc.vector.tensor_copy(
                            out=v_nat[b][:, cc : cc + w], in_=ps[:, :w]
                        )
                    else:
                        for h in range(H):
                            nc.vector.tensor_copy(
                                out=vlo[b][:, h, 0:32],
                                in_=ps[:, 32 * h : 32 * h + 32],
                            )

        # ---- unit generators (PSUM->SBUF copies alternate DVE / ACT) ----
        def cpy_st(st, out, in_):
            if st == 0:
                nc.vector.tensor_copy(out=out, in_=in_)
            else:
                nc.scalar.copy(out=out, in_=in_)

        def q_unit(b, qt, co):
            j, r = divmod(co, 2)
            ps = [
                psp.tile([P, 512], F32, tag="ps", name=f"psq{b}_{co}_{st}")
                for st in range(2)
            ]
            for ci in range(NCI_Q):
                for st in range(2):
                    nc.tensor.matmul(
                        ps[st],
                        wqc[j][:, ci, r * P : (r + 1) * P],
                        xt[b][:, ci, ST[st]],
                        start=(ci == 0),
                        stop=(ci == NCI_Q - 1),
                    )
            for st in range(2):
                cpy_st(st, qt[:, co, ST[st]], ps[st])

        def o_unit(b, at, co):
            ost = ostg.tile([P, S], F32, tag="ost", name=f"ost{b}_{co}")
            ps = [
                psp.tile([P, 512], F32, tag="ps", name=f"pso{b}_{co}_{st}")
                for st in range(2)
            ]
            for ci in range(NCI_Q):
                for st in range(2):
                    nc.tensor.matmul(
                        ps[st],
                        wo[:, ci, co * P : (co + 1) * P],
                        at[:, ci, ST[st]],
                        start=(ci == 0),
                        stop=(ci == NCI_Q - 1),
                    )
            for st in range(2):
                cpy_st(st, ost[:, ST[st]], ps[st])
            nc.sync.dma_start(
                out=otd_d.ap()[b, co * P : (co + 1) * P, :], in_=ost
            )

        def attn_front(b, qt, h):
            """Scores + exp for head h (both seq chunks)."""
            i, m = divmod(h, 4)
            exps = []
            for st in range(2):
                ps_s = psp.tile(
                    [SENC, 512], F32, tag="ps", name=f"sc{b}_{h}_{st}"
                )
                nc.tensor.matmul(
                    ps_s,
                    kt[h][:, b * SENC : (b + 1) * SENC],
                    qt[:, h, ST[st]],
                    start=True,
                    stop=False,
                )
                nc.tensor.matmul(
                    ps_s,
                    ktm[i][m][:, b * SENC : (b + 1) * SENC],
                    qt[:, H + i, ST[st]],
                    start=False,
                    stop=True,
                )
                ex = expp.tile(
                    [SENC, 512], BF16, tag="exps", name=f"ex{b}_{h}_{st}"
                )
                nc.scalar.activation(
                    out=ex, in_=ps_s, func=AF.Exp, scale=ATTN_SCALE
                )
                exps.append(ex)
            return exps

        def attn_back(b, at, h, exps):
            i, m = divmod(h, 4)
            ps_av, ps_lo = [], []
            for st in range(2):
                lo = psp.tile([33, 512], F32, tag="ps", name=f"lo{b}_{h}_{st}")
                nc.tensor.matmul(
                    lo, vlo[b][:, h, :], exps[st],
                    start=True, stop=True,
                )
                ps_lo.append(lo)
            for st in range(2):
                av = psp.tile([P, 512], F32, tag="ps", name=f"av{b}_{h}_{st}")
                nc.tensor.matmul(
                    av, v_nat[b][:, P * h : P * (h + 1)], exps[st],
                    start=True, stop=True,
                )
                ps_av.append(av)
            rec = recp.tile([1, S], BF16, tag="rec", name=f"rec{b}_{h}")
            bc = bcp.tile([P, S], BF16, tag="bc", name=f"bc{b}_{h}")
            lo = lost.tile([32, S], BF16, tag="lo", name=f"lost{b}_{h}")
            with nc.allow_low_precision(reason="bf16 softmax denominators"):
                for st in range(2):
                    nc.vector.reciprocal(
                        out=rec[:, ST[st]], in_=ps_lo[st][32:33, :]
                    )
                    nc.gpsimd.partition_broadcast(
                        bc[:, ST[st]], rec[:, ST[st]]
                    )
            for st in range(2):
                nc.vector.scalar_tensor_tensor(
                    out=at[:, h, ST[st]], in0=ps_av[st], scalar=1.0,
                    in1=bc[:, ST[st]], op0=MULT, op1=MULT,
                )
                nc.vector.scalar_tensor_tensor(
                    out=lo[:, ST[st]], in0=ps_lo[st][0:32, :], scalar=1.0,
                    in1=bc[0:32, ST[st]], op0=MULT, op1=MULT,
                )
            nc.sync.dma_start(
                out=at[32 * m : 32 * m + 32, H + i, :], in_=lo
            )

        # ---- mega-stream ----
        # batch 0.  Fronts lead their backs by TWO windows so the V
        # projection (emitted in window 2, when wv has landed) is done
        # before back(0,0); Q units keep a 2-window lead on their fronts.
        qt0 = apool.tile([P, NCO, S], BF16, tag="act", name="qt0")
        at0 = apool.tile([P, NCI_Q, S], BF16, tag="act", name="at0")
        qt1 = apool.tile([P, NCO, S], BF16, tag="act", name="qt1")
        _dummy = apool.tile([P, 1], BF16, tag="act", name="dummy")
        at1 = apool.tile([P, NCI_Q, S], BF16, tag="act", name="at1")

        for co in (H, H + 1, 0, 1):
            q_unit(0, qt0, co)
        fq = [attn_front(0, qt0, 0)]
        units0 = [lambda co=co: q_unit(0, qt0, co) for co in range(2, H)]
        units0 += [lambda co=co: q_unit(1, qt1, co) for co in (H, H + 1)]
        for h in range(H):
            if h < len(units0):
                units0[h]()
            if h + 1 < H:
                fq.append(attn_front(0, qt0, h + 1))
            if h == 2:
                v_proj()
            if h >= 2:
                attn_back(0, at0, h - 2, fq.pop(0))
        q_unit(1, qt1, 0)
        attn_back(0, at0, H - 2, fq.pop(0))
        q_unit(1, qt1, 1)
        attn_back(0, at0, H - 1, fq.pop(0))

        # batch 1, with O-proj b0 units thickening the windows (2 per window)
        fq = [attn_front(1, qt1, 0)]
        units1 = [lambda co=co: q_unit(1, qt1, co) for co in range(2, H)]
        units1 += [lambda co=co: o_unit(0, at0, co) for co in range(NCO)]
        ui = 0
        for h in range(H):
            until = min((h + 1) * 2, len(units1) - 2)
            while ui < until:
                units1[ui]()
                ui += 1
            nxt = attn_front(1, qt1, h + 1) if h + 1 < H else None
            attn_back(1, at1, h, fq.pop(0))
            if nxt is not None:
                fq.append(nxt)
        while ui < len(units1):
            units1[ui]()
            ui += 1

        # ---- O proj batch 1 ----
        for co in range(NCO):
            o_unit(1, at1, co)

    nc.compile()
    return nc


_NC_CACHE = []


def _get_nc():
    if not _NC_CACHE:
        _NC_CACHE.append(build())
    return _NC_CACHE[0]


def make_in_maps(hidden_states, encoder_hidden_states, Wq, Wk, Wv, Wo,
                 q_down, q_up, k_down, k_up, v_down, v_up, o_down, o_up):
    f64 = np.float64
    wq = Wq.astype(f64) + q_up.astype(f64) @ q_down.astype(f64)
    wk = Wk.astype(f64) + k_up.astype(f64) @ k_down.astype(f64)
    wv = Wv.astype(f64) + v_up.astype(f64) @ v_down.astype(f64)
    wo = Wo.astype(f64) + o_up.astype(f64) @ o_down.astype(f64)
    perm = head_perm()
    bf = ml_dtypes.bfloat16

    def wlay(w, nci):
        # [P, nci, C]: w[p, ci, c] = W[c, ci*128+p]
        return np.ascontiguousarray(
            w.T.reshape(nci, P, C).transpose(1, 0, 2).astype(bf)
        )

    wq_h = wlay(wq[perm, :], NCI_Q)
    wq_h = np.ascontiguousarray(
        wq_h.reshape(P, NCI_Q, 5, 256).transpose(2, 0, 1, 3)
    )
    wk_h = wlay(wk[perm, :], NCI_KV)
    wk_h = np.ascontiguousarray(
        wk_h.reshape(P, NCI_KV, 2, 640).transpose(2, 0, 1, 3)
    )
    wv_h = wlay(wv[perm, :], NCI_KV)
    wva_h = np.ascontiguousarray(wv_h[:, :, 0:512])
    wvb_h = np.ascontiguousarray(wv_h[:, :, 512:1280])
    wo_h = wlay(wo[:, perm], NCI_Q)

    in_maps = []
    for c in range(NCORES):
        hs = hidden_states[c * BPC : (c + 1) * BPC]  # [2, S, C]
        xt = np.ascontiguousarray(
            hs.transpose(0, 2, 1)
            .reshape(BPC, NCI_Q, P, S)
            .transpose(0, 2, 1, 3)
            .astype(bf)
        )
        enc = encoder_hidden_states[c * BPC : (c + 1) * BPC]  # [2, 77, 1024]
        et = np.zeros((CENC, EW), np.float32)
        for b in range(BPC):
            et[:, b * SENC : (b + 1) * SENC] = enc[b].T
        et = np.ascontiguousarray(
            et.reshape(NCI_KV, P, EW).transpose(1, 0, 2).astype(bf)
        )
        in_maps.append(
            {"xt": xt, "et": et, "wq": wq_h, "wk": wk_h, "wva": wva_h,
             "wvb": wvb_h, "wo": wo_h}
        )
    return in_maps


def kernel(hidden_states, encoder_hidden_states, Wq, Wk, Wv, Wo, bo,
           q_down, q_up, k_down, k_up, v_down, v_up, o_down, o_up):
    nc = _get_nc()
    in_maps = make_in_maps(
        hidden_states, encoder_hidden_states, Wq, Wk, Wv, Wo,
        q_down, q_up, k_down, k_up, v_down, v_up, o_down, o_up,
    )
    res = run_bass_kernel_spmd(nc, in_maps, list(range(NCORES)))
    out = np.empty((B, S, C), np.float32)
    for c in range(NCORES):
        ot = res.results[c]["otd"]  # [BPC, C, S]
        for b in range(BPC):
            out[c * BPC + b] = ot[b].T
    out += bo.astype(np.float32)[None, None, :]
    return out


# revision 20
# speedup vs baseline: 1.2013x; 1.2013x over previous
"""Trainium2 Bass kernel for LoRACrossAttnProcessor (v7, bf16, mega-stream).

Strategy:
- Host: fold LoRA (W_eff = W + up @ down, exact in f64), permute the qkv
  output channels (rows of Wq/Wk/Wv, cols of Wo) so each head owns one full
  128-row tile (tiles 0-7) plus a 32-row slice of the shared leftover tiles
  (8-9).  Pre-transpose to partition-major DRAM layouts (one contiguous
  chunk per SBUF partition) and cast to bf16 on the host.
- Shard: data-parallel over batch, 2 batch items per core, 8 cores.
- Device (per core, bf16 matmuls, fp32 PSUM):
    K.T tiles  = Wk_p @ E.T      [128, 154] per tile (both batches at once)
    V          = E @ Wv_p.T      [77, 1024] main + [77, 8, 33] leftover+ones
    Q.T        = Wq_p @ X.T      [128, 10, 1024] bf16 per batch
    per (batch, head, st-chunk):
      scores.T = kt/ktm MMs -> [77, 512] fp32 PSUM
      exps     = exp(scores.T * scale) -> bf16  (ACT only; kept FIFO-clean)
      A.T main = V[:, head-tile] @ exps          [128, 512]
      A.T left = [V_left | 1] @ exps -> [33, 512]; row 32 = sumexp
      recip -> bf16 (DVE), partition-broadcast bf16 (Pool), STT -> at
    O.T        = Wo_p @ A.T     [128, 1024] f32 staged, contiguous DMA out
- HBM-deadline-aware staging: wq is loaded in five column chunks (separate
  tiles) so Q-proj units can start ~20us in while later chunks stream; wv
  loads in two chunks with the V projection emitted mid-stream; attention
  backs trail their fronts by two windows so V is ready for head 0.
- Single mega-stream emission keeps the PE busy through both batches'
  attention; a shared 8-slot PSUM pool makes bank-reuse distance a full
  head-window.
- Host: transpose O.T -> O, add bo.
"""

import numpy as np
import ml_dtypes
from contextlib import ExitStack

import concourse.bass as bass
import concourse.mybir as mybir
import concourse.tile as tile
from concourse import bacc
from concourse.bass_utils import run_bass_kernel_spmd

F32 = mybir.dt.float32
BF16 = mybir.dt.bfloat16
AF = mybir.ActivationFunctionType
MULT = mybir.AluOpType.mult

H = 8
B, S, C = 16, 1024, 1280
SENC, CENC = 77, 1024
D = C // H  # 160
NCORES = 8
BPC = B // NCORES  # 2
P = 128
NCI_Q = C // P  # 10
NCI_KV = CENC // P  # 8
NCO = C // P  # 10
EW = 2 * SENC  # 154, both batches' encoder tokens side by side
ATTN_SCALE = 1.0 / float(np.sqrt(D))
ST = (slice(0, 512), slice(512, 1024))


def head_perm():
    """New channel order: head h gets rows [128h,128h+128) (its first 128
    dims) and rows [1024+32h, 1024+32h+32) (its last 32 dims)."""
    perm = []
    for h in range(H):
        perm.extend(range(D * h, D * h + P))
    for h in range(H):
        perm.extend(range(D * h + P, D * h + D))
    return np.asarray(perm)


def build():
    nc = bacc.Bacc("TRN2", target_bir_lowering=False, debug=False)
    xt_d = nc.dram_tensor("xt", [BPC, P, NCI_Q, S], BF16, kind="ExternalInput")
    et_d = nc.dram_tensor("et", [P, NCI_KV, EW], BF16, kind="ExternalInput")
    wk_d = nc.dram_tensor("wk", [P, NCI_KV, C], BF16, kind="ExternalInput")
    wva_d = nc.dram_tensor("wva", [P, NCI_KV, 512], BF16, kind="ExternalInput")
    wvb_d = nc.dram_tensor("wvb", [P, NCI_KV, 768], BF16, kind="ExternalInput")
    wq_d = nc.dram_tensor("wq", [5, P, NCI_Q, 256], BF16, kind="ExternalInput")
    wo_d = nc.dram_tensor("wo", [P, NCI_Q, C], BF16, kind="ExternalInput")
    otd_d = nc.dram_tensor("otd", [BPC, C, S], F32, kind="ExternalOutput")

    with tile.TileContext(nc) as tc, ExitStack() as ctx:
        wpool = ctx.enter_context(tc.tile_pool(name="wpool", bufs=1))  # wo only
        apool = ctx.enter_context(tc.tile_pool(name="apool", bufs=4))
        persist = ctx.enter_context(tc.tile_pool(name="persist", bufs=1))
        expp = ctx.enter_context(tc.tile_pool(name="expp", bufs=8))
        bcp = ctx.enter_context(tc.tile_pool(name="bcp", bufs=2))
        recp = ctx.enter_context(tc.tile_pool(name="recp", bufs=2))
        lost = ctx.enter_context(tc.tile_pool(name="lost", bufs=2))
        ostg = ctx.enter_context(tc.tile_pool(name="ostg", bufs=2))
        psp = ctx.enter_context(tc.tile_pool(name="psp", bufs=8, space="PSUM"))

        # ---- persistent buffers ----
        kt = [
            persist.tile([P, EW], BF16, tag=f"kt{t}", name=f"kt{t}")
            for t in range(H)
        ]
        ktm = [
            [
                persist.tile(
                    [P, EW], BF16, tag=f"ktm{i}_{m}", name=f"ktm{i}_{m}"
                )
                for m in range(4)
            ]
            for i in range(2)
        ]
        for i in range(2):
            for m in range(4):
                nc.vector.memset(ktm[i][m], 0.0)
        v_nat = [
            persist.tile([SENC, CENC], BF16, tag=f"vnat{b}", name=f"vnat{b}")
            for b in range(BPC)
        ]
        # [V_leftover(32) | ones] per head: row 32 of the A.T-leftover matmul
        # output is then the softmax denominator.
        vlo = [
            persist.tile([SENC, H, 33], BF16, tag=f"vlo{b}", name=f"vlo{b}")
            for b in range(BPC)
        ]
        for b in range(BPC):
            nc.vector.memset(vlo[b][:, :, 32:33], 1.0)
        et = persist.tile([P, NCI_KV, EW], BF16, tag="et")

        # ---- input DMAs, HBM-deadline order ----
        # sync ring:   wk, wq chunks (c4 first: Q cols for heads' tiles 8,9)
        # scalar ring: et, xt0, wva, wvb, xt1, wo
        wk = wpool.tile([P, NCI_KV, C], BF16, tag="w", name="wk")
        nc.sync.dma_start(out=wk, in_=wk_d.ap())
        wqc = [
            persist.tile([P, NCI_Q, 256], BF16, tag=f"wqc{j}", name=f"wqc{j}")
            for j in range(5)
        ]
        for j in (4, 0, 1, 2, 3):
            nc.sync.dma_start(out=wqc[j], in_=wq_d.ap()[j])
        nc.scalar.dma_start(out=et, in_=et_d.ap())
        xt = []
        for b in range(BPC):
            x = apool.tile([P, NCI_Q, S], BF16, tag="act", name=f"xt{b}")
            xt.append(x)
        nc.scalar.dma_start(out=xt[0], in_=xt_d.ap()[0])
        wva = persist.tile([P, NCI_KV, 512], BF16, tag="wva")
        nc.scalar.dma_start(out=wva, in_=wva_d.ap())
        wvb = persist.tile([P, NCI_KV, 768], BF16, tag="wvb")
        nc.scalar.dma_start(out=wvb, in_=wvb_d.ap())
        nc.scalar.dma_start(out=xt[1], in_=xt_d.ap()[1])

        # ---- K.T projection: both batches at once ----
        for t in range(NCO):
            ps = psp.tile([P, EW], F32, tag="ps", name=f"psk{t}")
            for ci in range(NCI_KV):
                nc.tensor.matmul(
                    ps,
                    wk[:, ci, t * P : (t + 1) * P],
                    et[:, ci, :],
                    start=(ci == 0),
                    stop=(ci == NCI_KV - 1),
                )
            if t < H:
                nc.vector.tensor_copy(out=kt[t], in_=ps)
            else:
                for m in range(4):
                    nc.vector.tensor_copy(
                        out=ktm[t - H][m][32 * m : 32 * m + 32, :],
                        in_=ps[32 * m : 32 * m + 32, :],
                    )

        wo = wpool.tile([P, NCI_Q, C], BF16, tag="w", name="wo")
        nc.scalar.dma_start(out=wo, in_=wo_d.ap())

        def v_proj():
            VCH = [(0, 512), (512, 512), (1024, 256)]
            VW = [
                lambda ci: wva[:, ci, 0:512],
                lambda ci: wvb[:, ci, 0:512],
                lambda ci: wvb[:, ci, 512:768],
            ]
            for b in range(BPC):
                for j, (cc, w) in enumerate(VCH):
                    ps = psp.tile(
                        [SENC, 512], F32, tag="ps", name=f"psv{b}_{j}"
                    )
                    for ci in range(NCI_KV):
                        nc.tensor.matmul(
                            ps[:, :w],
                            et[:, ci, b * SENC : (b + 1) * SENC],
                            VW[j](ci),
                            start=(ci == 0),
                            stop=(ci == NCI_KV - 1),
                        )
                    if j < 2:
                        nc.vector.tensor_copy(
                            out=v_nat[b][:, cc : cc + w], in_=ps[:, :w]
                        )
                    else:
                        for h in range(H):
                            nc.vector.tensor_copy(
                                out=vlo[b][:, h, 0:32],
                                in_=ps[:, 32 * h : 32 * h + 32],
                            )

        # ---- unit generators (PSUM->SBUF copies alternate DVE / ACT) ----
        def cpy_st(st, out, in_):
            if st == 0:
                nc.vector.tensor_copy(out=out, in_=in_)
            else:
                nc.scalar.copy(out=out, in_=in_)

        def q_unit(b, qt, co):
            j, r = divmod(co, 2)
            ps = [
                psp.tile([P, 512], F32, tag="ps", name=f"psq{b}_{co}_{st}")
                for st in range(2)
            ]
            for ci in range(NCI_Q):
                for st in range(2):
                    nc.tensor.matmul(
                        ps[st],
                        wqc[j][:, ci, r * P : (r + 1) * P],
                        xt[b][:, ci, ST[st]],
                        start=(ci == 0),
                        stop=(ci == NCI_Q - 1),
                    )
            for st in range(2):
                cpy_st(st, qt[:, co, ST[st]], ps[st])

        def o_unit(b, at, co):
            ost = ostg.tile([P, S], F32, tag="ost", name=f"ost{b}_{co}")
            ps = [
                psp.tile([P, 512], F32, tag="ps", name=f"pso{b}_{co}_{st}")
                for st in range(2)
            ]
            for ci in range(NCI_Q):
                for st in range(2):
                    nc.tensor.matmul(
                        ps[st],
                        wo[:, ci, co * P : (co + 1) * P],
                        at[:, ci, ST[st]],
                        start=(ci == 0),
                        stop=(ci == NCI_Q - 1),
                    )
            for st in range(2):
                cpy_st(st, ost[:, ST[st]], ps[st])
            nc.sync.dma_start(
                out=otd_d.ap()[b, co * P : (co + 1) * P, :], in_=ost
            )

        def attn_front(b, qt, h):
            """Scores + exp for head h (both seq chunks)."""
            i, m = divmod(h, 4)
            exps = []
            for st in range(2):
                ps_s = psp.tile(
                    [SENC, 512], F32, tag="ps", name=f"sc{b}_{h}_{st}"
                )
                nc.tensor.matmul(
                    ps_s,
                    kt[h][:, b * SENC : (b + 1) * SENC],
                    qt[:, h, ST[st]],
                    start=True,
                    stop=False,
                )
                nc.tensor.matmul(
                    ps_s,
                    ktm[i][m][:, b * SENC : (b + 1) * SENC],
                    qt[:, H + i, ST[st]],
                    start=False,
                    stop=True,
                )
                ex = expp.tile(
                    [SENC, 512], BF16, tag="exps", name=f"ex{b}_{h}_{st}"
                )
                nc.scalar.activation(
                    out=ex, in_=ps_s, func=AF.Exp, scale=ATTN_SCALE
                )
                exps.append(ex)
            return exps

        def attn_back(b, at, h, exps):
            i, m = divmod(h, 4)
            ps_av, ps_lo = [], []
            for st in range(2):
                lo = psp.tile([33, 512], F32, tag="ps", name=f"lo{b}_{h}_{st}")
                nc.tensor.matmul(
                    lo, vlo[b][:, h, :], exps[st],
                    start=True, stop=True,
                )
                ps_lo.append(lo)
            for st in range(2):
                av = psp.tile([P, 512], F32, tag="ps", name=f"av{b}_{h}_{st}")
                nc.tensor.matmul(
                    av, v_nat[b][:, P * h : P * (h + 1)], exps[st],
                    start=True, stop=True,
                )
                ps_av.append(av)
            rec = recp.tile([1, S], BF16, tag="rec", name=f"rec{b}_{h}")
            bc = bcp.tile([P, S], BF16, tag="bc", name=f"bc{b}_{h}")
            lo = lost.tile([32, S], BF16, tag="lo", name=f"lost{b}_{h}")
            with nc.allow_low_precision(reason="bf16 softmax denominators"):
                for st in range(2):
                    nc.vector.reciprocal(
                        out=rec[:, ST[st]], in_=ps_lo[st][32:33, :]
                    )
                    nc.gpsimd.partition_broadcast(
                        bc[:, ST[st]], rec[:, ST[st]]
                    )
            for st in range(2):
                nc.vector.scalar_tensor_tensor(
                    out=at[:, h, ST[st]], in0=ps_av[st], scalar=1.0,
                    in1=bc[:, ST[st]], op0=MULT, op1=MULT,
                )
                nc.vector.scalar_tensor_tensor(
                    out=lo[:, ST[st]], in0=ps_lo[st][0:32, :], scalar=1.0,
                    in1=bc[0:32, ST[st]], op0=MULT, op1=MULT,
                )
            nc.sync.dma_start(
                out=at[32 * m : 32 * m + 32, H + i, :], in_=lo
            )

        # ---- mega-stream ----
        # batch 0.  Fronts lead their backs by TWO windows so the V
        # projection (emitted in window 2, when wv has landed) is done
        # before back(0,0); Q units keep a 2-window lead on their fronts.
        qt0 = apool.tile([P, NCO, S], BF16, tag="act", name="qt0")
        at0 = apool.tile([P, NCI_Q, S], BF16, tag="act", name="at0")
        qt1 = apool.tile([P, NCO, S], BF16, tag="act", name="qt1")
        _dummy = apool.tile([P, 1], BF16, tag="act", name="dummy")
        at1 = apool.tile([P, NCI_Q, S], BF16, tag="act", name="at1")

        for co in (H, H + 1, 0, 1):
            q_unit(0, qt0, co)
        fq = [attn_front(0, qt0, 0)]
        units0 = [lambda co=co: q_unit(0, qt0, co) for co in range(2, H)]
        units0 += [lambda co=co: q_unit(1, qt1, co) for co in (H, H + 1)]
        for h in range(H):
            if h < len(units0):
                units0[h]()
            if h + 1 < H:
                fq.append(attn_front(0, qt0, h + 1))
            if h == 2:
                v_proj()
            if h >= 2:
                attn_back(0, at0, h - 2, fq.pop(0))
        q_unit(1, qt1, 0)
        attn_back(0, at0, H - 2, fq.pop(0))
        q_unit(1, qt1, 1)
        attn_back(0, at0, H - 1, fq.pop(0))

        # batch 1, with O-proj b0 units thickening the windows (2 per window)
        fq = [attn_front(1, qt1, 0)]
        units1 = [lambda co=co: q_unit(1, qt1, co) for co in range(2, H)]
        units1 += [lambda co=co: o_unit(0, at0, co) for co in range(NCO)]
        ui = 0
        for h in range(H):
            until = min((h + 1) * 2, len(units1) - 2)
            while ui < until:
                units1[ui]()
                ui += 1
            nxt = attn_front(1, qt1, h + 1) if h + 1 < H else None
            attn_back(1, at1, h, fq.pop(0))
            if nxt is not None:
                fq.append(nxt)
        while ui < len(units1):
            units1[ui]()
            ui += 1

        # ---- O proj batch 1 ----
        for co in range(NCO):
            o_unit(1, at1, co)

    nc.compile()
    return nc


_NC_CACHE = []


def _get_nc():
    if not _NC_CACHE:
        _NC_CACHE.append(build())
    return _NC_CACHE[0]


def make_in_maps(hidden_states, encoder_hidden_states, Wq, Wk, Wv, Wo,
                 q_down, q_up, k_down, k_up, v_down, v_up, o_down, o_up):
    f64 = np.float64
    wq = Wq.astype(f64) + q_up.astype(f64) @ q_down.astype(f64)
    wk = Wk.astype(f64) + k_up.astype(f64) @ k_down.astype(f64)
    wv = Wv.astype(f64) + v_up.astype(f64) @ v_down.astype(f64)
    wo = Wo.astype(f64) + o_up.astype(f64) @ o_down.astype(f64)
    perm = head_perm()
    bf = ml_dtypes.bfloat16

    def wlay(w, nci):
        # [P, nci, C]: w[p, ci, c] = W[c, ci*128+p]
        return np.ascontiguousarray(
            w.T.reshape(nci, P, C).transpose(1, 0, 2).astype(bf)
        )

    wq_h = wlay(wq[perm, :], NCI_Q)
    wq_h = np.ascontiguousarray(
        wq_h.reshape(P, NCI_Q, 5, 256).transpose(2, 0, 1, 3)
    )
    wk_h = wlay(wk[perm, :], NCI_KV)
    wv_h = wlay(wv[perm, :], NCI_KV)
    wva_h = np.ascontiguousarray(wv_h[:, :, 0:512])
    wvb_h = np.ascontiguousarray(wv_h[:, :, 512:1280])
    wo_h = wlay(wo[:, perm], NCI_Q)

    in_maps = []
    for c in range(NCORES):
        hs = hidden_states[c * BPC : (c + 1) * BPC]  # [2, S, C]
        xt = np.ascontiguousarray(
            hs.transpose(0, 2, 1)
            .reshape(BPC, NCI_Q, P, S)
            .transpose(0, 2, 1, 3)
            .astype(bf)
        )
        enc = encoder_hidden_states[c * BPC : (c + 1) * BPC]  # [2, 77, 1024]
        et = np.zeros((CENC, EW), np.float32)
        for b in range(BPC):
            et[:, b * SENC : (b + 1) * SENC] = enc[b].T
        et = np.ascontiguousarray(
            et.reshape(NCI_KV, P, EW).transpose(1, 0, 2).astype(bf)
        )
        in_maps.append(
            {"xt": xt, "et": et, "wq": wq_h, "wk": wk_h, "wva": wva_h,
             "wvb": wvb_h, "wo": wo_h}
        )
    return in_maps


def kernel(hidden_states, encoder_hidden_states, Wq, Wk, Wv, Wo, bo,
           q_down, q_up, k_down, k_up, v_down, v_up, o_down, o_up):
    nc = _get_nc()
    in_maps = make_in_maps(
        hidden_states, encoder_hidden_states, Wq, Wk, Wv, Wo,
        q_down, q_up, k_down, k_up, v_down, v_up, o_down, o_up,
    )
    res = run_bass_kernel_spmd(nc, in_maps, list(range(NCORES)))
    out = np.empty((B, S, C), np.float32)
    for c in range(NCORES):
        ot = res.results[c]["otd"]  # [BPC, C, S]
        for b in range(BPC):
            out[c * BPC + b] = ot[b].T
    out += bo.astype(np.float32)[None, None, :]
    return out
